# revision 11
# baseline (speedup 1.0000x reference)
"""Contextual loss kernel for Trainium2 (Bass/Tile), 8 NeuronCores.

Reference computation (per batch b, B=4, C=128, N=64*64=4096):
  mean_y[c] = spatial mean of feature_y
  fx,fy centered by mean_y; columns L2-normalized over channels
  S[n,m]    = <fxn[:,n], fyn[:,m]>           (cosine similarity)
  d = 1-S;  d_norm = d / (min_m d + 1e-3);  w = exp((1-d_norm)/h);  A = w/sum_m w
  CX[b] = mean_n max_m A;  loss = -log(CX)

Per-row identity used on device (with Smax = max_m S, c = 1/(h*(1-Smax+eps))):
  max_m A = 1 / sum_m exp(c*(S[m]-Smax))

x-normalization is folded into the activation scale: G = (x-mu).y_hat,
S = G/nx with nx = ||x-mu||+1e-10.  Row max of G gives Gmax; the exp pass
uses scale = 1/(H*((1+eps)*nx - Gmax)) and bias = -Gmax*scale, so the
x-normalize multiply pass is never materialized.

Sharding: 8 cores = 4 batches x 2 row-halves. Each core gets its half of
feature_x's rows ([2048,128]) plus the full feature_y ([4096,128]) of its
batch, computes sum_rows 1/r locally; host combines and takes -log.

Main loop per 128-row block: two interleaved passes with recompute
(pass1 max on PE+DVE, pass2 exp on PE+ACT, disjoint PSUM halves, one
block apart).  The per-block scalar chain (scale/bias) runs on GPSIMD.
Prologue is pipelined: DMA -> tree-mean (DVE) -> center (DVE+GPSIMD)
-> square (ACT) -> reduce (DVE) -> y-normalize (DVE+GPSIMD) ->
PE transposes -> ACT copies, in two y-halves so the main loop starts
while the later tiles are still being prepared.
"""

import numpy as np

import concourse.bacc as bacc
import concourse.bass as bass
import concourse.tile as tile
from concourse import masks, mybir
from concourse.bass_utils import run_bass_kernel_spmd

F32 = mybir.dt.float32
F32R = mybir.dt.float32r
AF = mybir.ActivationFunctionType
ALU = mybir.AluOpType
AX = mybir.AxisListType

B = 4
C = 128
N = 4096          # spatial positions per batch
ROWS = N // 2     # rows of S per core (x-half)
P = 128           # partitions
NYT = N // P      # 32 y tiles
NXT = ROWS // P   # 16 x tiles
CHUNK = 512       # matmul free dim (one PSUM bank)
QUART = 1024      # columns per PSUM quarter (2 banks)
NQ = N // QUART   # 4 quarters per row block
NRB = ROWS // P   # 16 row blocks per core

H_PARAM = 0.1
EPS_MIN = 0.001
EPS_NORM = 1e-10


def build_nc():
    nc = bacc.Bacc(None)
    fx = nc.declare_dram_parameter("fx", [ROWS, C], F32, isOutput=False)
    fy = nc.declare_dram_parameter("fy", [N, C], F32, isOutput=False)
    part = nc.declare_dram_parameter("part", [P, 1], F32, isOutput=True)

    fy_t = fy.rearrange("(i p) c -> p i c", p=P)   # [128, 32, 128]
    fx_t = fx.rearrange("(i p) c -> p i c", p=P)   # [128, 16, 128]

    with tile.TileContext(nc) as tc:
        with (
            tc.tile_pool(name="singles", bufs=1) as singles,
            tc.tile_pool(name="raw", bufs=1) as raw,
            tc.tile_pool(name="tmats", bufs=1) as tmats,
            tc.tile_pool(name="stat", bufs=3) as stat,
        ):
            # ---- constants ----
            identity = singles.tile([P, P], F32)
            masks.make_identity(nc, identity[:])
            ones_col = singles.tile([P, 1], F32)
            nc.vector.memset(ones_col[:], 1.0)
            ones_row = singles.tile([1, P], F32)
            nc.vector.memset(ones_row[:], 1.0)
            negh_col = singles.tile([P, 1], F32)    # -H (gpsimd chain const)
            nc.vector.memset(negh_col[:], -H_PARAM)
            invh_col = singles.tile([P, 1], F32)    # 1/H
            nc.vector.memset(invh_col[:], 1.0 / H_PARAM)

            mean_sb = singles.tile([1, C], F32)
            mean_bc = singles.tile([P, C], F32)
            nsy = singles.tile([P, NYT], F32)       # y squared norms
            nsx = singles.tile([P, NXT], F32)
            invy = singles.tile([P, NYT], F32)      # 1/(ny+eps)
            nxh = singles.tile([P, NXT], F32)       # H*(1+eps)*(nx+eps)
            scl_all = singles.tile([P, NRB], F32)   # c/nx per block
            nb_all = singles.tile([P, NRB], F32)    # -Gmax*scl per block
            rq_all = singles.tile([P, NRB, NQ], F32)

            # ---- load inputs (y first: the mean gates everything) ----
            ysp = raw.tile([P, NYT, C], F32)   # y, spatial-major tiles
            xsp = raw.tile([P, NXT, C], F32)
            for j in range(4):
                nc.sync.dma_start(
                    out=ysp[:, j * 8:(j + 1) * 8, :],
                    in_=fy_t[:, j * 8:(j + 1) * 8, :],
                )
            nc.scalar.dma_start(out=xsp[:, 0:8, :], in_=fx_t[:, 0:8, :])
            nc.scalar.dma_start(out=xsp[:, 8:16, :], in_=fx_t[:, 8:16, :])

            # ---- mean over y's spatial axis: pairwise tree on DVE ----
            # group sums start as soon as each DMA pair lands
            tr = raw.tile([P, 16, C], F32)
            colsum = singles.tile([P, C], F32)
            nc.vector.tensor_add(tr[:, 0:8], ysp[:, 0:8, :],
                                 ysp[:, 8:16, :])
            nc.vector.tensor_add(tr[:, 8:16], ysp[:, 16:24, :],
                                 ysp[:, 24:32, :])
            nc.vector.tensor_add(tr[:, 0:8], tr[:, 0:8], tr[:, 8:16])
            nc.vector.reduce_sum(
                colsum[:], tr[:, 0:8, :].rearrange("p t c -> p c t"),
                axis=AX.X)

            # partition-sum via PE, scale by 1/N, broadcast to all partitions
            with tc.tile_pool(name="ps_mean", bufs=1,
                              space=bass.MemorySpace.PSUM) as ps_mean_pool:
                ps_mean = ps_mean_pool.tile([1, C], F32)
                nc.tensor.matmul(ps_mean[:], ones_col[:], colsum[:],
                                 start=True, stop=True)
                nc.scalar.mul(mean_sb[:], ps_mean[:], 1.0 / N)
            with tc.tile_pool(name="ps_bc", bufs=1,
                              space=bass.MemorySpace.PSUM) as ps_bc_pool:
                ps_bc = ps_bc_pool.tile([P, C], F32)
                nc.tensor.matmul(ps_bc[:], ones_row[:], mean_sb[:],
                                 start=True, stop=True)
                nc.vector.tensor_copy(mean_bc[:], ps_bc[:])

            mean_g = mean_bc[:].rearrange("p (u c) -> p u c", u=1)

            # ---- center / norms / normalize, pipelined in two y-halves ----
            sq = raw.tile([P, NYT, C], F32)     # squares scratch (y)
            sqx = raw.tile([P, NXT, C], F32)    # squares scratch (x)
            H0 = NYT // 2

            def mb(k):
                return mean_g.broadcast_to([P, k, C])

            # y half 0 + x on DVE (fast), y half 1 on GPSIMD, all in
            # 8-tile groups so cross-engine deps stay precise
            yg = [ysp[:, g * 8:(g + 1) * 8, :] for g in range(4)]
            sg = [sq[:, g * 8:(g + 1) * 8, :] for g in range(4)]
            nc.vector.tensor_sub(yg[0], yg[0], mb(8))
            nc.vector.tensor_sub(yg[1], yg[1], mb(8))
            nc.gpsimd.tensor_sub(yg[2], yg[2], mb(8))
            nc.gpsimd.tensor_sub(yg[3], yg[3], mb(8))
            nc.vector.tensor_sub(xsp[:], xsp[:], mb(NXT))

            for g in (0, 1):
                nc.scalar.activation(out=sg[g], in_=yg[g], func=AF.Square)
                nc.vector.reduce_sum(nsy[:, g * 8:(g + 1) * 8], sg[g],
                                     axis=AX.X)
            nc.scalar.activation(out=sqx[:], in_=xsp[:], func=AF.Square)
            nc.vector.reduce_sum(nsx[:], sqx[:], axis=AX.X)
            for g in (2, 3):
                nc.scalar.activation(out=sg[g], in_=yg[g], func=AF.Square)
                nc.vector.reduce_sum(nsy[:, g * 8:(g + 1) * 8], sg[g],
                                     axis=AX.X)

            # invy = 1/(sqrt(nsy)+eps); nxh = H*(1+eps_min)*(sqrt(nsx)+eps)
            sdy = singles.tile([P, NYT], F32)
            sdx = singles.tile([P, NXT], F32)
            nc.scalar.activation(sdy[:, :H0], nsy[:, :H0], AF.Sqrt)
            nc.vector.tensor_scalar_add(sdy[:, :H0], sdy[:, :H0], EPS_NORM)
            nc.vector.reciprocal(invy[:, :H0], sdy[:, :H0])
            nc.scalar.activation(sdx[:], nsx[:], AF.Sqrt)
            nc.vector.tensor_scalar(
                out=nxh[:], in0=sdx[:], scalar1=EPS_NORM,
                scalar2=H_PARAM * (1.0 + EPS_MIN),
                op0=ALU.add, op1=ALU.mult)
            nc.scalar.activation(sdy[:, H0:], nsy[:, H0:], AF.Sqrt)
            nc.vector.tensor_scalar_add(sdy[:, H0:], sdy[:, H0:], EPS_NORM)
            nc.vector.reciprocal(invy[:, H0:], sdy[:, H0:])

            # normalize y columns (x stays unnormalized; folded into scale)
            iy = invy[:].rearrange("p (t u) -> p t u", u=1)
            for g in (0, 1):
                nc.vector.tensor_mul(
                    yg[g], yg[g],
                    iy[:, g * 8:(g + 1) * 8].broadcast_to([P, 8, C]))
            for g in (2, 3):
                nc.gpsimd.tensor_mul(
                    yg[g], yg[g],
                    iy[:, g * 8:(g + 1) * 8].broadcast_to([P, 8, C]))

            # ---- transposes to channel-major; ACT copies PSUM->SBUF ----
            ytc = [tmats.tile([P, CHUNK], F32R, tag=f"ytc{j}",
                              name=f"ytc{j}")
                   for j in range(N // CHUNK)]     # y: [C, m] chunks
            xt = tmats.tile([P, ROWS], F32R)       # x: [C, n]

            # order: x batch 0 and y half 0 first so block 0 unblocks early
            batches = []      # (src view, dst ap)
            for q in range(4):                     # x tiles 0..15
                batches.append((xsp[:, q * 4:(q + 1) * 4, :],
                                xt[:, q * 4 * P:(q + 1) * 4 * P]))
            xb = batches
            yb = [(ysp[:, q * 4:(q + 1) * 4, :], ytc[q][:])
                  for q in range(8)]
            order = [xb[0]] + yb[0:4] + [xb[1]] + yb[4:8] + xb[2:4]

            with tc.tile_pool(name="ps_tr", bufs=4,
                              space=bass.MemorySpace.PSUM) as ps_tr_pool:
                for src, dst in order:
                    pst = ps_tr_pool.tile([P, 4 * P], F32, tag="pst")
                    for k in range(4):
                        nc.tensor.transpose(pst[:, k * P:(k + 1) * P],
                                            src[:, k, :], identity[:])
                    nc.scalar.copy(dst, pst[:])

            # ---- main loop: two interleaved passes with recompute ----
            def pass1(rb, pool):
                lhs = xt[:, rb * P:(rb + 1) * P]
                mxq = stat.tile([P, NQ], F32, tag="mxq", name="mxq")
                for q in range(NQ):
                    ps = pool.tile([P, QUART], F32, tag="p1", name="ps1")
                    for j in range(2):
                        nc.tensor.matmul(
                            ps[:, j * CHUNK:(j + 1) * CHUNK],
                            lhs, ytc[2 * q + j][:],
                            start=True, stop=True)
                    nc.vector.reduce_max(mxq[:, q:q + 1], ps[:], axis=AX.X)
                gmax = stat.tile([P, 1], F32, tag="gmax", name="gmax")
                nc.vector.reduce_max(gmax[:], mxq[:], axis=AX.X)
                # hg = -H*Gmax ; tden = hg + nxh ; scl = 1/tden ;
                # nb = (hg*scl)/H = -Gmax*scl          (all gpsimd, tiny)
                hg = stat.tile([P, 1], F32, tag="hg", name="hg")
                tden = stat.tile([P, 1], F32, tag="tden", name="tden")
                nc.gpsimd.tensor_mul(hg[:], gmax[:], negh_col[:])
                nc.gpsimd.tensor_add(tden[:], hg[:], nxh[:, rb:rb + 1])
                nc.vector.reciprocal(scl_all[:, rb:rb + 1], tden[:])
                hs = stat.tile([P, 1], F32, tag="hs", name="hs")
                nc.gpsimd.tensor_mul(hs[:], hg[:], scl_all[:, rb:rb + 1])
                nc.gpsimd.tensor_mul(nb_all[:, rb:rb + 1], hs[:],
                                     invh_col[:])

            def pass2(rb, pool):
                lhs = xt[:, rb * P:(rb + 1) * P]
                for q in range(NQ):
                    ps = pool.tile([P, QUART], F32, tag="p2", name="ps2")
                    for j in range(2):
                        nc.tensor.matmul(
                            ps[:, j * CHUNK:(j + 1) * CHUNK],
                            lhs, ytc[2 * q + j][:],
                            start=True, stop=True)
                    nc.scalar.activation(
                        out=ps[:], in_=ps[:], func=AF.Exp,
                        bias=nb_all[:, rb:rb + 1],
                        scale=scl_all[:, rb:rb + 1],
                        accum_out=rq_all[:, rb, q:q + 1])

            with (
                tc.tile_pool(name="ps_p1", bufs=2,
                             space=bass.MemorySpace.PSUM) as pool1,
                tc.tile_pool(name="ps_p2", bufs=2,
                             space=bass.MemorySpace.PSUM) as pool2,
            ):
                for rb in range(NRB + 1):
                    if rb < NRB:
                        pass1(rb, pool1)
                    if rb >= 1:
                        pass2(rb - 1, pool2)

            # ---- tail: r per block, 1/r, reduce, write out ----
            r_all = singles.tile([P, NRB], F32)
            nc.vector.reduce_sum(r_all[:], rq_all[:], axis=AX.X)
            invr_all = singles.tile([P, NRB], F32)
            nc.vector.reciprocal(invr_all[:], r_all[:])
            part_sb = singles.tile([P, 1], F32)
            nc.vector.reduce_sum(part_sb[:], invr_all[:], axis=AX.X)
            nc.sync.dma_start(out=part[:], in_=part_sb[:])

    nc.compile()
    return nc


_NC_CACHE = None


def _get_nc():
    global _NC_CACHE
    if _NC_CACHE is None:
        _NC_CACHE = build_nc()
    return _NC_CACHE


def _in_maps(feature_x, feature_y):
    fx = np.ascontiguousarray(
        np.asarray(feature_x, dtype=np.float32).reshape(B, N, C))
    fy = np.ascontiguousarray(
        np.asarray(feature_y, dtype=np.float32).reshape(B, N, C))
    maps = []
    for core in range(8):
        b, h = divmod(core, 2)
        maps.append({
            "fx": np.ascontiguousarray(fx[b, h * ROWS:(h + 1) * ROWS, :]),
            "fy": fy[b],
        })
    return maps


def _combine(results):
    sums = [float(np.asarray(r["part"], dtype=np.float64).sum())
            for r in results]
    loss = np.empty(B, dtype=np.float64)
    for b in range(B):
        cx = (sums[2 * b] + sums[2 * b + 1]) / N
        loss[b] = -np.log(cx)
    return loss.astype(np.float32)


def kernel(feature_x, feature_y):
    nc = _get_nc()
    res = run_bass_kernel_spmd(nc, _in_maps(feature_x, feature_y),
                               core_ids=list(range(8)))
    return _combine(res.results)


def kernel_traced(feature_x, feature_y, **kwargs):
    """Like kernel() but with tracing; returns (loss, BassKernelResults)."""
    nc = _get_nc()
    res = run_bass_kernel_spmd(nc, _in_maps(feature_x, feature_y),
                               core_ids=list(range(8)), trace=True, **kwargs)
    return _combine(res.results), res
